# revision 12
# baseline (speedup 1.0000x reference)
"""Trainium2 Bass kernel for nn_BaseConvPlus (dense_cnn).

Math: the reference computes
  1) kernel[b,c,:,:]  = global-mean of a depthwise 3x3 conv of x          -> [B,CIN,3,3]
  2) win  = einsum(kernel, w_in) + b_in ; wout = einsum(kernel, w_out)
  3) y[b] = conv2d(x[b], weight[b]) with weight[b,o,i] = win[b,i]*wout[b,o]

Two identities make this memory-bound:
  * mean(conv(x, k)) over HxW only needs the total sum, edge-row/col sums
    and corner pixels of each channel (zero 'SAME' padding) - no conv.
  * weight[b] is rank-1 across (o, i): y[b,o] = wout[b,o] * z[b] with
    z[b] = sum_i conv2d(x[b,i], win[b,i]).  We fuse the outer product into
    the matmul weights: for each tap j, lhsT_j[(b,i),(b',o)] =
    win[b,i,j]*wout[b,o]*delta(b,b') and accumulate 9 shifted matmuls into
    y_psum[(b,o), pixels] directly (4 samples packed into the K=128
    contraction).

Sharding: pure data parallel, 4 samples per core on 8 cores.
"""
import sys

sys.path.insert(0, "/opt/trn_rl_repo")

from contextlib import ExitStack

import numpy as np

import concourse.bacc as bacc
import concourse.bass as bass
import concourse.mybir as mybir
import concourse.tile as tile
from concourse.bass_utils import run_bass_kernel_spmd

B, CIN, COUT, KS, H, W = 32, 32, 32, 3, 192, 192
NCORES = 8
BC = B // NCORES          # 4 samples per core
P = BC * CIN              # 128 partitions = (sample, channel)
HP, WP = H + 2, W + 2     # 194: zero-padded image
NPIX = HP * WP            # 37636 padded pixels
LR = 16                   # rows per input DMA chunk
NCHUNK = H // LR          # 12
R = 2                     # output rows per conv tile (N = R*W = 384 <= 512)
NT = H // R               # 96 conv tiles
GT = 8                    # conv tiles per output DMA (16 rows, 1.5 MiB)
NG = NT // GT             # 12 output DMAs
F32 = mybir.dt.float32
BF16 = mybir.dt.bfloat16
AX = mybir.AxisListType
OP = mybir.AluOpType


def build_program(nc: bass.Bass) -> None:
    x_d = nc.dram_tensor("x", [BC, CIN, H, W], F32, kind="ExternalInput").ap()
    wk9_d = nc.dram_tensor("wk9", [P, 81], F32, kind="ExternalInput").ap()
    lwin_d = nc.dram_tensor("lwin", [P, P], F32, kind="ExternalInput").ap()
    brep_d = nc.dram_tensor("brep", [P, 1], F32, kind="ExternalInput").ap()
    wo9_d = nc.dram_tensor("wo9", [P, 9 * P], F32, kind="ExternalInput").ap()
    mask_d = nc.dram_tensor("mask", [P, P], F32, kind="ExternalInput").ap()
    ident_d = nc.dram_tensor("ident", [P, P], F32, kind="ExternalInput").ap()
    y_d = nc.dram_tensor("y", [BC, COUT, H, W], F32, kind="ExternalOutput").ap()

    xf = x_d.rearrange("b c h w -> (b c) (h w)")       # [128, 36864]
    yf = y_d.rearrange("b o h w -> (b o) (h w)")       # [128, 36864]

    with tile.TileContext(nc) as tc, ExitStack() as ctx:
        const = ctx.enter_context(tc.tile_pool(name="const", bufs=1))
        ypool = ctx.enter_context(tc.tile_pool(name="ysb", bufs=2))
        psum = ctx.enter_context(tc.tile_pool(name="psum", bufs=4, space="PSUM"))
        psum_s = ctx.enter_context(tc.tile_pool(name="psum_s", bufs=1, space="PSUM"))

        xpad = const.tile([P, NPIX], BF16)
        wk9 = const.tile([P, 81], F32)
        lwin = const.tile([P, P], F32)
        brep = const.tile([P, 1], F32)
        wo9 = const.tile([P, 9 * P], F32)
        mask = const.tile([P, P], F32)
        ident = const.tile([P, P], F32)
        scr = const.tile([P, 64], F32)     # 0:T 1:RF 2:RL 3:CF 4:CL 5..8 corners
        #                                    16..27 T partials, 28..39 CF, 40..51 CL
        svec = const.tile([P, 9], F32)
        srep = const.tile([P, 81], F32)
        t81 = const.tile([P, 81], F32)
        kern = const.tile([P, 9], F32)
        winsb = const.tile([P, 9], F32)
        woutsb = const.tile([P, 1], F32)
        ltmp = const.tile([P, P], F32)
        lhsd = const.tile([P, 9 * P], BF16)

        nc.sync.dma_start(out=wk9[:], in_=wk9_d)
        nc.sync.dma_start(out=lwin[:], in_=lwin_d)
        nc.sync.dma_start(out=brep[:], in_=brep_d)
        nc.sync.dma_start(out=wo9[:], in_=wo9_d)
        nc.sync.dma_start(out=mask[:], in_=mask_d)
        nc.sync.dma_start(out=ident[:], in_=ident_d)

        x3 = xpad[:].rearrange("p (r c) -> p r c", c=WP)   # [128, 194, 194]

        # zero the padding ring
        nc.vector.memset(x3[:, 0, :], 0.0)
        nc.vector.memset(x3[:, HP - 1, :], 0.0)
        nc.vector.memset(x3[:, 1:1 + H, 0], 0.0)
        nc.vector.memset(x3[:, 1:1 + H, WP - 1], 0.0)

        # load x (fp32 -> bf16 cast in DMA); per-chunk partial sums depend on
        # exactly one DMA each (walrus allows few sync waits per instruction)
        for i in range(NCHUNK):
            h0 = i * LR
            rows = x3[:, h0 + 1:h0 + 1 + LR, 1:1 + W]
            nc.gpsimd.dma_start(out=rows, in_=xf[:, h0 * W:(h0 + LR) * W])
            nc.vector.reduce_sum(out=scr[:, 16 + i:17 + i], in_=rows, axis=AX.XY)
            nc.vector.reduce_sum(
                out=scr[:, 28 + i:29 + i], in_=x3[:, h0 + 1:h0 + 1 + LR, 1],
                axis=AX.X)
            nc.vector.reduce_sum(
                out=scr[:, 40 + i:41 + i], in_=x3[:, h0 + 1:h0 + 1 + LR, W],
                axis=AX.X)

        # total and edge sums (padded coords: real (h,w) at (h+1, w+1))
        nc.vector.reduce_sum(out=scr[:, 0:1], in_=scr[:, 16:16 + NCHUNK], axis=AX.X)
        nc.vector.reduce_sum(out=scr[:, 1:2], in_=x3[:, 1, 1:1 + W], axis=AX.X)      # row 0
        nc.vector.reduce_sum(out=scr[:, 2:3], in_=x3[:, H, 1:1 + W], axis=AX.X)      # row H-1
        nc.vector.reduce_sum(out=scr[:, 3:4], in_=scr[:, 28:28 + NCHUNK], axis=AX.X)  # col 0
        nc.vector.reduce_sum(out=scr[:, 4:5], in_=scr[:, 40:40 + NCHUNK], axis=AX.X)  # col W-1
        nc.vector.tensor_copy(scr[:, 5:6], x3[:, 1, 1:2])          # x[0,0]
        nc.vector.tensor_copy(scr[:, 6:7], x3[:, 1, W:W + 1])      # x[0,W-1]
        nc.vector.tensor_copy(scr[:, 7:8], x3[:, H, 1:2])          # x[H-1,0]
        nc.vector.tensor_copy(scr[:, 8:9], x3[:, H, W:W + 1])      # x[H-1,W-1]

        # S[m] for tap m=(dy,dx): T - excluded row - excluded col + corner
        T = scr[:, 0:1]
        rf, rl, cf, cl = scr[:, 1:2], scr[:, 2:3], scr[:, 3:4], scr[:, 4:5]
        c00, c0l, cl0, cll = scr[:, 5:6], scr[:, 6:7], scr[:, 7:8], scr[:, 8:9]
        stt = nc.vector.scalar_tensor_tensor
        stt(svec[:, 0:1], T, rl, cl, op0=OP.subtract, op1=OP.subtract)
        nc.vector.tensor_add(svec[:, 0:1], svec[:, 0:1], cll)
        nc.vector.tensor_sub(svec[:, 1:2], T, rl)
        stt(svec[:, 2:3], T, rl, cf, op0=OP.subtract, op1=OP.subtract)
        nc.vector.tensor_add(svec[:, 2:3], svec[:, 2:3], cl0)
        nc.vector.tensor_sub(svec[:, 3:4], T, cl)
        nc.vector.tensor_copy(svec[:, 4:5], T)
        nc.vector.tensor_sub(svec[:, 5:6], T, cf)
        stt(svec[:, 6:7], T, rf, cl, op0=OP.subtract, op1=OP.subtract)
        nc.vector.tensor_add(svec[:, 6:7], svec[:, 6:7], c0l)
        nc.vector.tensor_sub(svec[:, 7:8], T, rf)
        stt(svec[:, 8:9], T, rf, cf, op0=OP.subtract, op1=OP.subtract)
        nc.vector.tensor_add(svec[:, 8:9], svec[:, 8:9], c00)

        # kernel[p, j] = sum_m wk9[p, j*9+m] * S[p, m]   (wk9 pre-scaled 1/(H*W))
        for j in range(9):
            nc.vector.tensor_copy(srep[:, j * 9:(j + 1) * 9], svec[:])
        nc.vector.tensor_mul(t81[:], wk9[:], srep[:])
        nc.vector.reduce_sum(
            out=kern[:], in_=t81[:].rearrange("p (j m) -> p j m", m=9), axis=AX.X)

        # win = blockdiag(w_in.T) @ kernel + b_in
        win_ps = psum_s.tile([P, 9], F32)
        nc.tensor.matmul(win_ps[:], lhsT=lwin[:], rhs=kern[:], start=True, stop=True)
        nc.vector.tensor_scalar_add(winsb[:], win_ps[:], brep[:])

        # wout[(b,o)] = sum_j blockdiag(w_out[:,:,j].T) @ kernel[:, j]
        wout_ps = psum_s.tile([P, 1], F32)
        for j in range(9):
            nc.tensor.matmul(
                wout_ps[:], lhsT=wo9[:, j * P:(j + 1) * P], rhs=kern[:, j:j + 1],
                start=(j == 0), stop=(j == 8))
        nc.vector.tensor_copy(woutsb[:], wout_ps[:])

        # wrep[(b,i),(b',o)] = wout[b',o] * delta(b,b'):
        # scale the block mask by wout per partition, then PE-transpose.
        nc.vector.tensor_scalar_mul(ltmp[:], mask[:], woutsb[:])
        wrep_ps = psum_s.tile([P, P], F32)
        nc.tensor.transpose(wrep_ps[:], ltmp[:], ident[:])

        # fused conv weights lhsT_j = wrep * win[:, j] (bf16)
        for j in range(9):
            nc.vector.tensor_scalar_mul(
                lhsd[:, j * P:(j + 1) * P], wrep_ps[:], winsb[:, j:j + 1])

        # main conv: 9 shifted matmuls accumulate y[(b,o), 2 rows x 192]
        for g in range(NG):
            ysb = ypool.tile([P, GT * R * W], F32)
            for t in range(GT):
                h0 = (g * GT + t) * R
                yps = psum.tile([P, R * W], F32)
                for j in range(9):
                    ky, kx = divmod(j, 3)
                    nc.tensor.matmul(
                        yps[:],
                        lhsT=lhsd[:, j * P:(j + 1) * P],
                        rhs=x3[:, h0 + ky:h0 + ky + R, kx:kx + W],
                        start=(j == 0), stop=(j == 8))
                nc.scalar.copy(out=ysb[:, t * R * W:(t + 1) * R * W], in_=yps[:])
            nc.sync.dma_start(
                out=yf[:, g * GT * R * W:(g + 1) * GT * R * W], in_=ysb[:])


def host_tables(wk, w_in, b_in, w_out):
    wk9 = np.tile(
        (wk.reshape(CIN, 9, 9) / float(H * W)).reshape(CIN, 81).astype(np.float32),
        (BC, 1))
    lwin = np.kron(np.eye(BC, dtype=np.float32), w_in.T.astype(np.float32))
    brep = np.tile(b_in.astype(np.float32), BC)[:, None]
    w9 = w_out.reshape(COUT, CIN, 9).astype(np.float32)
    wo9 = np.concatenate(
        [np.kron(np.eye(BC, dtype=np.float32), w9[:, :, j].T) for j in range(9)],
        axis=1)
    mask = np.kron(np.eye(BC, dtype=np.float32), np.ones((CIN, CIN), np.float32))
    ident = np.eye(P, dtype=np.float32)
    return {
        "wk9": np.ascontiguousarray(wk9, np.float32),
        "lwin": np.ascontiguousarray(lwin, np.float32),
        "brep": np.ascontiguousarray(brep, np.float32),
        "wo9": np.ascontiguousarray(wo9, np.float32),
        "mask": np.ascontiguousarray(mask, np.float32),
        "ident": np.ascontiguousarray(ident, np.float32),
    }


_CACHE: dict = {}


def _get_program() -> bass.Bass:
    if "nc" not in _CACHE:
        nc = bacc.Bacc(
            trn_type="TRN2", target_bir_lowering=False, debug=False,
            num_devices=NCORES)
        build_program(nc)
        nc.compile()
        _CACHE["nc"] = nc
    return _CACHE["nc"]


def kernel(x, wk, w_in, b_in, w_out, _trace=False, _trace_kwargs=None):
    x = np.ascontiguousarray(np.asarray(x), np.float32)
    tables = host_tables(np.asarray(wk), np.asarray(w_in), np.asarray(b_in),
                         np.asarray(w_out))
    nc = _get_program()
    in_maps = [
        {"x": np.ascontiguousarray(x[c * BC:(c + 1) * BC]), **tables}
        for c in range(NCORES)
    ]
    res = run_bass_kernel_spmd(
        nc, in_maps, core_ids=list(range(NCORES)),
        trace=_trace, **(_trace_kwargs or {}))
    y = np.concatenate([res.results[c]["y"] for c in range(NCORES)], axis=0)
    if _trace:
        return y, res
    return y


if __name__ == "__main__":
    rng = np.random.default_rng(0)
    inputs = {
        "x": rng.standard_normal((B, CIN, H, W), np.float32),
        "wk": rng.standard_normal((CIN * 9, 1, 3, 3)).astype(np.float32) * 0.05,
        "w_in": rng.standard_normal((CIN, CIN)).astype(np.float32) * 0.05,
        "b_in": rng.standard_normal((CIN,)).astype(np.float32) * 0.05,
        "w_out": rng.standard_normal((COUT, CIN, 3, 3)).astype(np.float32) * 0.05,
    }
    y = kernel(**inputs)
    print("y", y.shape, y.dtype, float(np.abs(y).max()))
